# revision 1
# baseline (speedup 1.0000x reference)
"""Trainium2 Bass kernel for nn_EnhancedSAGEModel (GNN message passing).

Strategy: node-partition across 8 cores (dst-sharding). Each core owns 6250
nodes (padded to 6272 = 49 windows of 128). Per layer:
  - mean-aggregation via dma_gather of h rows (bf16) + one-hot selector
    matmuls on the tensor engine (selector built on-chip by DVE is_equal,
    scaled by 1/deg), accumulating in PSUM (fp32)
  - dense SAGE update + BatchNorm (global stats via AllReduce) + ReLU +
    residual skip, computed feature-major ([256, nodes]) in fp32
  - h_new transposed back to row-major (PE transpose) and AllGathered so
    every core holds the full bf16 h table for the next layer's gather.
Final MLP + log_softmax fused per 512-node chunk.
"""

import os
import sys

sys.path.insert(0, "/opt/trn_rl_repo")

STAGE = int(os.environ.get("KSTAGE", "99"))
KMLP = os.environ.get("KMLP", "full")
NLAYERS = int(os.environ.get("KLAYERS", str(4)))

import numpy as np
import ml_dtypes

import concourse.bass as bass
import concourse.bacc as bacc
import concourse.mybir as mybir
import concourse.tile as tile
from concourse import bass_utils
from concourse.alu_op_type import AluOpType

F32 = mybir.dt.float32
BF16 = mybir.dt.bfloat16
I16 = mybir.dt.int16

N, E, DIN, H, L, DOUT = 50000, 800000, 256, 256, 4, 2
EPS = 1e-5
NCORES = 8
PPC = N // NCORES            # 6250 real nodes per core
NW = 49                      # windows per core
PN = NW * 128                # 6272 padded nodes per core
PAD = PN - PPC               # 22
NPAD = NCORES * PN           # 50176 padded global rows
HALF = NPAD // 2             # 25088
KBUF = 11                    # gather tiles per dma_gather call
CHUNK = 512
CHUNKS = [(i * CHUNK, CHUNK) for i in range(PN // CHUNK)] + [(PN - PN % CHUNK, PN % CHUNK)]
CHUNKS = [(s, w) for (s, w) in CHUNKS if w > 0]
NCH = len(CHUNKS)
INV_N = 1.0 / N

# bias column layout in the packed [128, NBCOL] bias tensor
def _bias_cols():
    cols = {}
    c = 0
    for lay in range(L):
        for nm in ("bl", "bng", "bnb", "skb"):
            cols[(nm, lay)] = c
            c += 2
    cols[("inb", 0)] = c; c += 2
    cols[("b1", 0)] = c; c += 4
    cols[("b2", 0)] = c; c += 2
    cols[("outb", 0)] = c; c += 1
    return cols, c

BIAS_COLS, NBCOL = _bias_cols()


def _wrap_idx(arr):
    """int array -> [128, len/16] int16 wrapped layout, replicated x8."""
    n = len(arr)
    assert n % 16 == 0
    w = arr.reshape(n // 16, 16).T.astype(np.int16)  # [16, n/16]
    return np.tile(w, (8, 1))


def _tile_cols(arr, ntiles):
    """[ntiles*128] -> [128, ntiles], column t = tile t values."""
    return np.ascontiguousarray(arr.reshape(ntiles, 128).T)


def _pack_vec(v):
    """[256] -> [128, 2] (col pt = v[pt*128:(pt+1)*128])."""
    return np.ascontiguousarray(np.asarray(v, np.float32).reshape(2, 128).T)


def plan_edges(edge_index):
    """Build the static per-core aggregation plan from the edge list."""
    src = edge_index[0].astype(np.int64)
    dst = edge_index[1].astype(np.int64)
    deg = np.bincount(dst, minlength=N).astype(np.float64)
    deginv_n = (1.0 / np.clip(deg, 1.0, None)).astype(np.float32)

    core = dst // PPC
    dloc = dst - core * PPC
    win = dloc // 128
    dwin = dloc % 128
    gsrc = src + PAD * (src // PPC)       # position in padded global table
    half = (gsrc >= HALF).astype(np.int64)
    idxval = gsrc - half * HALF
    dgi = deginv_n[dst]

    order = np.lexsort((src, half, win, core))
    core_s, win_s, half_s = core[order], win[order], half[order]
    idx_s, dwin_s, dgi_s = idxval[order], dwin[order], dgi[order]

    # group boundaries for (core, win, half)
    key = (core_s * NW + win_s) * 2 + half_s
    bounds = np.searchsorted(key, np.arange(NCORES * NW * 2 + 1))
    cnt = (bounds[1:] - bounds[:-1]).reshape(NCORES, NW, 2)
    tiles_needed = -(-cnt // 128)                      # ceil
    TA = tiles_needed[:, :, 0].max(axis=0)
    TB = tiles_needed[:, :, 1].max(axis=0)
    for w in range(NW):
        if TA[w] + TB[w] == 0:
            TA[w] = 1
    TA_tot, TB_tot = int(TA.sum()), int(TB.sum())
    T_tot = TA_tot + TB_tot

    cores = []
    for c in range(NCORES):
        idxA = np.zeros(TA_tot * 128, np.int64)
        idxB = np.zeros(TB_tot * 128, np.int64)
        dstv = np.zeros(T_tot * 128, np.float32)
        dgv = np.zeros(T_tot * 128, np.float32)
        pa = pb = pt = 0
        for w in range(NW):
            for h, (idxarr, tcount, p0) in enumerate(
                ((idxA, TA[w], pa), (idxB, TB[w], pb))
            ):
                g = (c * NW + w) * 2 + h
                s, e = bounds[g], bounds[g + 1]
                n = e - s
                assert n <= tcount * 128
                idxarr[p0 : p0 + n] = idx_s[s:e]
                dstv[pt : pt + n] = dwin_s[s:e]
                dgv[pt : pt + n] = dgi_s[s:e]
                pt += tcount * 128
                if h == 0:
                    pa += tcount * 128
                else:
                    pb += tcount * 128
        cores.append(
            dict(
                gidxA=_wrap_idx(idxA),
                gidxB=_wrap_idx(idxB),
                dstv=_tile_cols(dstv, T_tot),
                dgv=_tile_cols(dgv, T_tot),
            )
        )
    return dict(TA=[int(x) for x in TA], TB=[int(x) for x in TB],
                TA_tot=TA_tot, TB_tot=TB_tot, T_tot=T_tot, cores=cores)


def build_program(TA, TB, TA_tot, TB_tot, T_tot):
    nc = bacc.Bacc("TRN2", target_bir_lowering=False, debug=False,
                   num_devices=NCORES)
    RG = [list(range(NCORES))]

    # ---- DRAM I/O ----
    d_xt = nc.dram_tensor("xt", [2 * 128, PN], F32, kind="ExternalInput")
    d_gidxA = nc.dram_tensor("gidxA", [128, TA_tot * 8], I16, kind="ExternalInput")
    d_gidxB = nc.dram_tensor("gidxB", [128, TB_tot * 8], I16, kind="ExternalInput")
    d_dstv = nc.dram_tensor("dstv", [128, T_tot], F32, kind="ExternalInput")
    d_dgv = nc.dram_tensor("dgv", [128, T_tot], F32, kind="ExternalInput")
    d_iota = nc.dram_tensor("iota", [128, 128], BF16, kind="ExternalInput")
    d_ident = nc.dram_tensor("ident", [128, 128], F32, kind="ExternalInput")
    d_bias = nc.dram_tensor("bias", [128, NBCOL], F32, kind="ExternalInput")
    d_wl = nc.dram_tensor("wl", [L * 256, 256], BF16, kind="ExternalInput")
    d_wr = nc.dram_tensor("wr", [L * 256, 256], F32, kind="ExternalInput")
    d_sk = nc.dram_tensor("sk", [L * 256, 256], F32, kind="ExternalInput")
    d_inw = nc.dram_tensor("inw", [256, 256], F32, kind="ExternalInput")
    d_w1 = nc.dram_tensor("w1", [256, 512], F32, kind="ExternalInput")
    d_w2 = nc.dram_tensor("w2", [512, 256], F32, kind="ExternalInput")
    d_ow = nc.dram_tensor("ow", [256, DOUT], F32, kind="ExternalInput")
    d_perm = nc.dram_tensor("perm", [DOUT, DOUT], F32, kind="ExternalInput")
    d_out = nc.dram_tensor("out", [DOUT, PN], F32, kind="ExternalOutput")

    re_tp = lambda ap: ap.rearrange("(t p) o -> p t o", p=128)

    with tile.TileContext(nc) as tc:
        with (
            tc.tile_pool(name="big", bufs=1) as big,
            tc.tile_pool(name="wts", bufs=1) as wts,
            tc.tile_pool(name="gth", bufs=3) as gth,
            tc.tile_pool(name="sel", bufs=4) as selp,
            tc.tile_pool(name="scr", bufs=3) as scr,
            tc.tile_pool(name="sml", bufs=1) as sml,
            tc.tile_pool(name="psA", bufs=2, space="PSUM") as psA,
            tc.tile_pool(name="psB", bufs=2, space="PSUM") as psB,
            tc.tile_pool(name="psT", bufs=2, space="PSUM") as psT,
            tc.tile_pool(name="dram", bufs=1, space="DRAM") as dram,
        ):
            # persistent SBUF
            X = big.tile([128, 2, PN], F32, name="X")          # h (feature-major)
            Y = big.tile([128, 2, PN], F32, name="Y")          # z / scratch
            dstv = big.tile([128, T_tot], F32, name="dstv")
            dgv = big.tile([128, T_tot], F32, name="dgv")
            iota = big.tile([128, 128], BF16, name="iota")
            ident = big.tile([128, 128], F32, name="ident")
            biases = big.tile([128, NBCOL], F32, name="biases")
            zsum = big.tile([128, 2, NCH], F32, name="zsum")
            zsq = big.tile([128, 2, NCH], F32, name="zsq")
            arpack = big.tile([128, 4], F32, name="arpack")
            arsb = big.tile([128, 4], F32, name="arsb")
            musb = big.tile([128, 2], F32, name="musb")
            varsb = big.tile([128, 2], F32, name="varsb")
            scsb = big.tile([128, 2], F32, name="scsb")
            shsb = big.tile([128, 2], F32, name="shsb")
            tmp2 = big.tile([128, 2], F32, name="tmp2")

            # DRAM internals
            hfull = dram.tile([NPAD, 256], BF16, name="hfull")
            ag_in = dram.tile([PN, 256], BF16, name="ag_in")
            ar_in = dram.tile([128, 4], F32, name="ar_in")
            ar_out = dram.tile([128, 4], F32, name="ar_out")

            gixA = big.tile([128, TA_tot * 8], I16, name="gixA")
            gixB = big.tile([128, TB_tot * 8], I16, name="gixB")
            nc.sync.dma_start(gixA[:], d_gidxA.ap())
            nc.sync.dma_start(gixB[:], d_gidxB.ap())
            nc.sync.dma_start(dstv[:], d_dstv.ap())
            nc.sync.dma_start(dgv[:], d_dgv.ap())
            nc.sync.dma_start(iota[:], d_iota.ap())
            nc.sync.dma_start(ident[:], d_ident.ap())
            nc.sync.dma_start(biases[:], d_bias.ap())

            def bias_ap(nm, lay, pt, npart=128):
                col = BIAS_COLS[(nm, lay if nm in ("bl", "bng", "bnb", "skb") else 0)]
                return biases[0:npart, col + pt : col + pt + 1]

            def tail_transpose_ag(lay):
                """X (feature-major fp32) -> stag bf16 row-major -> ag_in -> AG."""
                stag = big.tile([128, NW, 256], BF16, name=f"stag{lay}",
                                tag="aggbuf")
                for nt in range(NW):
                    for fh in range(2):
                        pst = psT.tile([128, 128], F32, name=f"pst{lay}_{nt}_{fh}",
                                       tag="pst")
                        nc.tensor.transpose(
                            pst[:], X[:, fh, nt * 128 : (nt + 1) * 128], ident[:]
                        )
                        nc.vector.tensor_copy(
                            stag[:, nt, fh * 128 : (fh + 1) * 128], pst[:]
                        )
                nc.sync.dma_start(
                    ag_in.rearrange("(w p) f -> p w f", p=128), stag[:]
                )
                nc.gpsimd.collective_compute(
                    "AllGather", mybir.AluOpType.bypass, replica_groups=RG,
                    ins=[ag_in.opt()], outs=[hfull.opt()],
                )

            # ---------------- phase 0: input projection ----------------
            inw = wts.tile([128, 2, 256], F32, name="inw", tag="wA")
            nc.sync.dma_start(inw[:], re_tp(d_inw.ap()))
            nc.sync.dma_start(Y[:, :, :], d_xt.ap().rearrange("(t p) n -> p t n", p=128))
            for pt in range(2):
                for ci, (s, w) in enumerate(CHUNKS):
                    ps = psB.tile([128, CHUNK], F32, name=f"ps0_{pt}_{ci}", tag="psz")
                    for fi in range(2):
                        nc.tensor.matmul(
                            ps[:, :w], inw[:, fi, pt * 128 : (pt + 1) * 128],
                            Y[:, fi, s : s + w], start=(fi == 0), stop=(fi == 1),
                        )
                    nc.scalar.activation(
                        X[:, pt, s : s + w], ps[:, :w],
                        mybir.ActivationFunctionType.Relu,
                        bias=bias_ap("inb", 0, pt),
                    )
            tail_transpose_ag(-1)

            # ---------------- conv layers ----------------
            for lay in range(L if STAGE >= 1 else 0):
                aggT = big.tile([128, 2, PN], BF16, name=f"aggT{lay}", tag="aggbuf")
                wl = wts.tile([128, 2, 256], BF16, name=f"wl{lay}", tag="wC")
                wr = wts.tile([128, 2, 256], F32, name=f"wr{lay}", tag="wA")
                sk = wts.tile([128, 2, 256], F32, name=f"sk{lay}", tag="wB")
                nc.sync.dma_start(wl[:], re_tp(d_wl.ap()[lay * 256 : (lay + 1) * 256, :]))
                nc.sync.dma_start(wr[:], re_tp(d_wr.ap()[lay * 256 : (lay + 1) * 256, :]))
                nc.sync.dma_start(sk[:], re_tp(d_sk.ap()[lay * 256 : (lay + 1) * 256, :]))

                # --- aggregation ---
                gbufs = {}   # (half, call) -> tile

                def ensure_call(hf, t0, lay=lay):
                    callno = t0 // KBUF
                    if (hf, callno) in gbufs:
                        return gbufs[(hf, callno)]
                    tot = TA_tot if hf == 0 else TB_tot
                    kk = min(KBUF, tot - callno * KBUF)
                    gix = gixA if hf == 0 else gixB
                    gb = gth.tile([128, kk, 256], BF16, name=f"gb{lay}_{hf}_{callno}",
                                  tag=f"gb{hf}")
                    src_ap = hfull[0:HALF, :] if hf == 0 else hfull[HALF:NPAD, :]
                    nc.gpsimd.dma_gather(gb[:], src_ap,
                                         gix[:, callno * KBUF * 8 : (callno * KBUF + kk) * 8],
                                         kk * 128, kk * 128, 256,
                                         single_packet=False)
                    gbufs[(hf, callno)] = gb
                    return gb

                a = b = t = 0
                for w in range(NW):
                    ntile_w = TA[w] + TB[w]
                    pw = [
                        psA.tile([128, 128], F32, name=f"pw{lay}_{w}_{fh}", tag=f"pw{fh}")
                        for fh in range(2)
                    ]
                    tl = 0
                    for hf, cnt in ((0, TA[w]), (1, TB[w])):
                        for _ in range(cnt):
                            gb = ensure_call(hf, a if hf == 0 else b)
                            r = (a if hf == 0 else b) % KBUF
                            sel = selp.tile([128, 128], BF16, name=f"sel{lay}_{t}",
                                            tag="sel")
                            nc.vector.tensor_scalar(
                                sel[:], iota[:], dstv[:, t : t + 1], dgv[:, t : t + 1],
                                AluOpType.is_equal, AluOpType.mult,
                            )
                            for fh in range(2):
                                nc.tensor.matmul(
                                    pw[fh][:], gb[:, r, fh * 128 : (fh + 1) * 128],
                                    sel[:], start=(tl == 0), stop=(tl == ntile_w - 1),
                                )
                            if hf == 0:
                                a += 1
                            else:
                                b += 1
                            t += 1
                            tl += 1
                    for fh in range(2):
                        nc.vector.tensor_copy(
                            aggT[:, fh, w * 128 : (w + 1) * 128], pw[fh][:]
                        )

                # --- dense z = wl@aggT + wr@hT + bl ; stats ---
                for pt in range(2 if STAGE >= 2 else 0):
                    for ci, (s, w) in enumerate(CHUNKS):
                        ps = psB.tile([128, CHUNK], F32, name=f"psz{lay}_{pt}_{ci}",
                                      tag="psz")
                        for fi in range(2):
                            nc.tensor.matmul(
                                ps[:, :w], wl[:, fi, pt * 128 : (pt + 1) * 128],
                                aggT[:, fi, s : s + w], start=(fi == 0), stop=False,
                            )
                        for fi in range(2):
                            nc.tensor.matmul(
                                ps[:, :w], wr[:, fi, pt * 128 : (pt + 1) * 128],
                                X[:, fi, s : s + w], start=False, stop=(fi == 1),
                            )
                        vw = min(w, max(0, PPC - s))  # valid (non-pad) columns
                        nc.scalar.activation(
                            Y[:, pt, s : s + w], ps[:, :w],
                            mybir.ActivationFunctionType.Identity,
                            bias=bias_ap("bl", lay, pt),
                            accum_out=None,
                        )
                        sq = scr.tile([128, CHUNK], F32, name=f"sq{lay}_{pt}_{ci}",
                                      tag="sq", bufs=6)
                        if vw > 0:
                            nc.scalar.activation(
                                sq[:, :vw], Y[:, pt, s : s + vw],
                                mybir.ActivationFunctionType.Identity,
                                accum_out=zsum[:, pt, ci : ci + 1],
                            )
                            nc.scalar.activation(
                                sq[:, :vw], Y[:, pt, s : s + vw],
                                mybir.ActivationFunctionType.Square,
                                accum_out=zsq[:, pt, ci : ci + 1],
                            )
                        else:
                            nc.vector.memset(zsum[:, pt, ci : ci + 1], 0.0)
                            nc.vector.memset(zsq[:, pt, ci : ci + 1], 0.0)

                for pt in range(2 if STAGE >= 2 else 0):
                    nc.vector.reduce_sum(arpack[:, pt : pt + 1], zsum[:, pt, :],
                                         axis=mybir.AxisListType.X)
                    nc.vector.reduce_sum(arpack[:, 2 + pt : 3 + pt], zsq[:, pt, :],
                                         axis=mybir.AxisListType.X)
                if STAGE >= 2:
                    nc.sync.dma_start(ar_in[:], arpack[:])
                    nc.gpsimd.collective_compute(
                        "AllReduce", mybir.AluOpType.add, replica_groups=RG,
                        ins=[ar_in.opt()], outs=[ar_out.opt()],
                    )
                    nc.sync.dma_start(arsb[:], ar_out[:])

                    # BN scale/shift
                    nc.vector.tensor_scalar_mul(musb[:], arsb[:, 0:2], INV_N)
                    nc.vector.tensor_scalar_mul(varsb[:], arsb[:, 2:4], INV_N)
                    nc.vector.tensor_tensor(tmp2[:], musb[:], musb[:], AluOpType.mult)
                    nc.vector.tensor_tensor(varsb[:], varsb[:], tmp2[:], AluOpType.subtract)
                    nc.vector.tensor_scalar_add(varsb[:], varsb[:], EPS)
                    nc.scalar.sqrt(varsb[:], varsb[:])
                    nc.vector.reciprocal(varsb[:], varsb[:])
                    nc.vector.tensor_tensor(
                        scsb[:], biases[:, BIAS_COLS[("bng", lay)] : BIAS_COLS[("bng", lay)] + 2],
                        varsb[:], AluOpType.mult,
                    )
                    nc.vector.tensor_tensor(tmp2[:], musb[:], scsb[:], AluOpType.mult)
                    nc.vector.tensor_tensor(
                        shsb[:], biases[:, BIAS_COLS[("bnb", lay)] : BIAS_COLS[("bnb", lay)] + 2],
                        tmp2[:], AluOpType.subtract,
                    )

                # skip + normalize + residual add
                for ci, (s, w) in enumerate(CHUNKS if STAGE >= 2 else []):
                    pss = []
                    for pt in range(2):
                        psk = psB.tile([128, CHUNK], F32, name=f"psk{lay}_{pt}_{ci}",
                                       tag="psz")
                        for fi in range(2):
                            nc.tensor.matmul(
                                psk[:, :w], sk[:, fi, pt * 128 : (pt + 1) * 128],
                                X[:, fi, s : s + w], start=(fi == 0), stop=(fi == 1),
                            )
                        pss.append(psk)
                    for pt in range(2):
                        nc.scalar.activation(
                            Y[:, pt, s : s + w], Y[:, pt, s : s + w],
                            mybir.ActivationFunctionType.Relu,
                            bias=shsb[:, pt : pt + 1], scale=scsb[:, pt : pt + 1],
                        )
                        nc.vector.scalar_tensor_tensor(
                            X[:, pt, s : s + w], Y[:, pt, s : s + w],
                            bias_ap("skb", lay, pt), pss[pt][:, :w],
                            AluOpType.add, AluOpType.add,
                        )

                if lay < L - 1 and STAGE >= 2:
                    tail_transpose_ag(lay)

            # ---------------- MLP head + log_softmax ----------------
            w1 = wts.tile([128, 2, 512], F32, name="w1", tag="wA")
            w2 = wts.tile([128, 4, 256], F32, name="w2", tag="wB")
            ow = wts.tile([128, 2, DOUT], F32, name="ow", tag="wC")
            nc.sync.dma_start(w1[:], re_tp(d_w1.ap()))
            nc.sync.dma_start(w2[:], re_tp(d_w2.ap()))
            nc.sync.dma_start(ow[:], re_tp(d_ow.ap()))
            perm = big.tile([DOUT, DOUT], F32, name="perm")
            nc.sync.dma_start(perm[:], d_perm.ap())

            for ci, (s, w) in enumerate(CHUNKS if STAGE >= 3 else []):
                m1 = gth.tile([128, 4, 512], F32, name=f"m1_{ci}", tag="gb0")
                for q in range(4):
                    ps1 = psB.tile([128, CHUNK], F32, name=f"ps1_{ci}_{q}", tag="psz")
                    for fi in range(2):
                        nc.tensor.matmul(
                            ps1[:, :w], w1[:, fi, q * 128 : (q + 1) * 128],
                            X[:, fi, s : s + w], start=(fi == 0), stop=(fi == 1),
                        )
                    nc.scalar.activation(
                        m1[:, q, :w], ps1[:, :w],
                        mybir.ActivationFunctionType.Relu, bias=bias_ap("b1", 0, q),
                    )
                m2 = gth.tile([128, 2, 512], F32, name=f"m2_{ci}", tag="gb1")
                for pt in range(2):
                    ps2 = psB.tile([128, CHUNK], F32, name=f"ps2_{ci}_{pt}", tag="psz")
                    for q in range(4):
                        nc.tensor.matmul(
                            ps2[:, :w], w2[:, q, pt * 128 : (pt + 1) * 128],
                            m1[:, q, :w], start=(q == 0), stop=(q == 3),
                        )
                    nc.scalar.activation(
                        m2[:, pt, :w], ps2[:, :w],
                        mybir.ActivationFunctionType.Identity, bias=bias_ap("b2", 0, pt),
                    )
                if KMLP == "m2":
                    nc.sync.dma_start(d_out.ap()[:, s : s + w], m2[0:DOUT, 0, :w])
                    continue
                psl = psB.tile([DOUT, CHUNK], F32, name=f"psl_{ci}", tag="psz")
                for fi in range(2):
                    nc.tensor.matmul(
                        psl[:, :w], ow[:, fi, :], m2[:, fi, :w],
                        start=(fi == 0), stop=(fi == 1),
                    )
                lg = scr.tile([DOUT, CHUNK], F32, name=f"lg_{ci}", tag="sq", bufs=6)
                nc.scalar.activation(
                    lg[:, :w], psl[:, :w],
                    mybir.ActivationFunctionType.Identity,
                    bias=bias_ap("outb", 0, 0, npart=DOUT),
                )
                if KMLP == "logits":
                    nc.sync.dma_start(d_out.ap()[:, s : s + w], lg[:, :w])
                    continue
                psw = psB.tile([DOUT, CHUNK], F32, name=f"psw_{ci}", tag="psz")
                nc.tensor.matmul(psw[:, :w], perm[:], lg[:, :w], start=True, stop=True)
                if KMLP == "perm":
                    lsw0 = scr.tile([DOUT, CHUNK], F32, name=f"lsw0_{ci}", tag="sq", bufs=6)
                    nc.vector.tensor_copy(lsw0[:, :w], psw[:, :w])
                    nc.sync.dma_start(d_out.ap()[:, s : s + w], lsw0[:, :w])
                    continue
                lsw = scr.tile([DOUT, CHUNK], F32, name=f"lsw_{ci}", tag="sq", bufs=6)
                nc.vector.tensor_copy(lsw[:, :w], psw[:, :w])
                mx = scr.tile([DOUT, CHUNK], F32, name=f"mx_{ci}", tag="sq", bufs=6)
                nc.vector.tensor_tensor(mx[:, :w], lg[:, :w], lsw[:, :w], AluOpType.max)
                nc.vector.tensor_tensor(lg[:, :w], lg[:, :w], mx[:, :w], AluOpType.subtract)
                nc.vector.tensor_tensor(lsw[:, :w], lsw[:, :w], mx[:, :w], AluOpType.subtract)
                if KMLP == "mx":
                    nc.sync.dma_start(d_out.ap()[:, s : s + w], lsw[:, :w])
                    continue
                ex = scr.tile([DOUT, CHUNK], F32, name=f"ex_{ci}", tag="sq", bufs=6)
                nc.scalar.activation(ex[:, :w], lg[:, :w],
                                     mybir.ActivationFunctionType.Exp)
                nc.scalar.activation(lsw[:, :w], lsw[:, :w],
                                     mybir.ActivationFunctionType.Exp)
                if KMLP == "exp":
                    nc.sync.dma_start(d_out.ap()[:, s : s + w], ex[:, :w])
                    continue
                nc.vector.tensor_tensor(ex[:, :w], ex[:, :w], lsw[:, :w], AluOpType.add)
                ln_ = scr.tile([DOUT, CHUNK], F32, name=f"ln_{ci}", tag="sq", bufs=6)
                nc.scalar.activation(ln_[:, :w], ex[:, :w],
                                     mybir.ActivationFunctionType.Ln)
                ot_ = scr.tile([DOUT, CHUNK], F32, name=f"ot_{ci}", tag="sq", bufs=6)
                nc.vector.tensor_tensor(ot_[:, :w], lg[:, :w], ln_[:, :w], AluOpType.subtract)
                nc.sync.dma_start(d_out.ap()[:, s : s + w], ot_[:, :w])

            if STAGE < 3:
                nc.sync.dma_start(d_out.ap(), X[0:DOUT, 0, :])

    nc.compile()
    return nc


_CACHE = {}


def kernel(**inputs):
    inputs = {k: np.asarray(v) for k, v in inputs.items()}
    edge_index = inputs["edge_index"]
    key = hash(edge_index.tobytes())
    if key not in _CACHE:
        plan = plan_edges(edge_index)
        nc = build_program(plan["TA"], plan["TB"], plan["TA_tot"],
                           plan["TB_tot"], plan["T_tot"])
        _CACHE.clear()
        _CACHE[key] = (plan, nc)
    plan, nc = _CACHE[key]

    x = inputs["x"].astype(np.float32)
    # shared (replicated) tensors
    bias = np.zeros((128, NBCOL), np.float32)
    for lay in range(L):
        bias[:, BIAS_COLS[("bl", lay)] : BIAS_COLS[("bl", lay)] + 2] = _pack_vec(inputs["conv_bl"][lay])
        bias[:, BIAS_COLS[("bng", lay)] : BIAS_COLS[("bng", lay)] + 2] = _pack_vec(inputs["bn_g"][lay])
        bias[:, BIAS_COLS[("bnb", lay)] : BIAS_COLS[("bnb", lay)] + 2] = _pack_vec(inputs["bn_b"][lay])
        bias[:, BIAS_COLS[("skb", lay)] : BIAS_COLS[("skb", lay)] + 2] = _pack_vec(inputs["skip_b"][lay])
    bias[:, BIAS_COLS[("inb", 0)] : BIAS_COLS[("inb", 0)] + 2] = _pack_vec(inputs["in_b"])
    b1c = BIAS_COLS[("b1", 0)]
    bias[:, b1c : b1c + 4] = np.asarray(inputs["mlp_b1"], np.float32).reshape(4, 128).T
    bias[:, BIAS_COLS[("b2", 0)] : BIAS_COLS[("b2", 0)] + 2] = _pack_vec(inputs["mlp_b2"])
    bias[0:DOUT, BIAS_COLS[("outb", 0)]] = np.asarray(inputs["out_b"], np.float32)

    shared = dict(
        iota=np.tile(np.arange(128, dtype=np.float32), (128, 1)).astype(ml_dtypes.bfloat16),
        ident=np.eye(128, dtype=np.float32),
        bias=bias,
        wl=np.concatenate([np.ascontiguousarray(inputs["conv_wl"][i].T) for i in range(L)],
                          axis=0).astype(ml_dtypes.bfloat16),
        wr=np.concatenate([np.ascontiguousarray(inputs["conv_wr"][i].T) for i in range(L)],
                          axis=0).astype(np.float32),
        sk=np.concatenate([np.ascontiguousarray(inputs["skip_w"][i].T) for i in range(L)],
                          axis=0).astype(np.float32),
        inw=np.ascontiguousarray(inputs["in_w"].T.astype(np.float32)),
        w1=np.ascontiguousarray(inputs["mlp_w1"].T.astype(np.float32)),
        w2=np.ascontiguousarray(inputs["mlp_w2"].T.astype(np.float32)),
        ow=np.ascontiguousarray(inputs["out_w"].T.astype(np.float32)),
        perm=np.array([[0.0, 1.0], [1.0, 0.0]], np.float32),
    )

    in_maps = []
    for c in range(NCORES):
        xt = np.zeros((256, PN), np.float32)
        xt[:, :PPC] = x[c * PPC : (c + 1) * PPC].T
        m = dict(shared)
        m["xt"] = xt
        m.update(plan["cores"][c])
        in_maps.append(m)

    res = bass_utils.run_bass_kernel_spmd(nc, in_maps, core_ids=list(range(NCORES)))
    out = np.empty((N, DOUT), np.float32)
    for c in range(NCORES):
        out[c * PPC : (c + 1) * PPC] = res.results[c]["out"][:, :PPC].T
    return out



# revision 7
# speedup vs baseline: 1.1104x; 1.1104x over previous
"""Trainium2 Bass kernel for nn_EnhancedSAGEModel (GNN message passing).

Strategy: node-partition across 8 cores (dst-sharding). Each core owns 6250
nodes (padded to 6272 = 49 windows of 128). Per layer:
  - mean-aggregation via dma_gather of h rows (bf16) + one-hot selector
    matmuls on the tensor engine (selector built on-chip by DVE is_equal,
    scaled by 1/deg), accumulating in PSUM (fp32)
  - dense SAGE update + BatchNorm (global stats via AllReduce) + ReLU +
    residual skip, computed feature-major ([256, nodes]) in fp32
  - h_new transposed back to row-major (PE transpose) and AllGathered so
    every core holds the full bf16 h table for the next layer's gather.
Final MLP + log_softmax fused per 512-node chunk.
"""

import os
import sys

sys.path.insert(0, "/opt/trn_rl_repo")

STAGE = int(os.environ.get("KSTAGE", "99"))
KMLP = os.environ.get("KMLP", "full")
NLAYERS = int(os.environ.get("KLAYERS", str(4)))
KNOAG = os.environ.get("KNOAG", "") == "1"
KNOGATHER = os.environ.get("KNOGATHER", "") == "1"
KNOSEL = os.environ.get("KNOSEL", "") == "1"

import numpy as np
import ml_dtypes

import concourse.bass as bass
import concourse.bacc as bacc
import concourse.mybir as mybir
import concourse.tile as tile
from concourse import bass_utils
from concourse.alu_op_type import AluOpType

F32 = mybir.dt.float32
BF16 = mybir.dt.bfloat16
I16 = mybir.dt.int16

N, E, DIN, H, L, DOUT = 50000, 800000, 256, 256, 4, 2
EPS = 1e-5
NCORES = 8
PPC = N // NCORES            # 6250 real nodes per core
NW = 49                      # windows per core
PN = NW * 128                # 6272 padded nodes per core
PAD = PN - PPC               # 22
NPAD = NCORES * PN           # 50176 padded global rows
HALF = NPAD // 2             # 25088
KBUF = 11                    # gather tiles per dma_gather call
CHUNK = 512
CHUNKS = [(i * CHUNK, CHUNK) for i in range(PN // CHUNK)] + [(PN - PN % CHUNK, PN % CHUNK)]
CHUNKS = [(s, w) for (s, w) in CHUNKS if w > 0]
NCH = len(CHUNKS)
INV_N = 1.0 / N

# bias column layout in the packed [128, NBCOL] bias tensor
def _bias_cols():
    cols = {}
    c = 0
    for lay in range(L):
        for nm in ("bl", "bng", "bnb", "skb"):
            cols[(nm, lay)] = c
            c += 2
    cols[("inb", 0)] = c; c += 2
    cols[("b1", 0)] = c; c += 4
    cols[("b2", 0)] = c; c += 2
    cols[("outb", 0)] = c; c += 1
    return cols, c

BIAS_COLS, NBCOL = _bias_cols()


def _wrap_idx(arr):
    """int array -> [128, len/16] int16 wrapped layout, replicated x8."""
    n = len(arr)
    assert n % 16 == 0
    w = arr.reshape(n // 16, 16).T.astype(np.int16)  # [16, n/16]
    return np.tile(w, (8, 1))


def _tile_cols(arr, ntiles):
    """[ntiles*128] -> [128, ntiles], column t = tile t values."""
    return np.ascontiguousarray(arr.reshape(ntiles, 128).T)


def _pack_vec(v):
    """[256] -> [128, 2] (col pt = v[pt*128:(pt+1)*128])."""
    return np.ascontiguousarray(np.asarray(v, np.float32).reshape(2, 128).T)


def plan_edges(edge_index):
    """Build the static per-core aggregation plan from the edge list."""
    src = edge_index[0].astype(np.int64)
    dst = edge_index[1].astype(np.int64)
    deg = np.bincount(dst, minlength=N).astype(np.float64)
    deginv_n = (1.0 / np.clip(deg, 1.0, None)).astype(np.float32)

    core = dst // PPC
    dloc = dst - core * PPC
    win = dloc // 128
    dwin = dloc % 128
    gsrc = src + PAD * (src // PPC)       # position in padded global table
    half = (gsrc >= HALF).astype(np.int64)
    idxval = gsrc - half * HALF
    dgi = deginv_n[dst]

    order = np.lexsort((src, half, win, core))
    core_s, win_s, half_s = core[order], win[order], half[order]
    idx_s, dwin_s, dgi_s = idxval[order], dwin[order], dgi[order]

    # group boundaries for (core, win, half)
    key = (core_s * NW + win_s) * 2 + half_s
    bounds = np.searchsorted(key, np.arange(NCORES * NW * 2 + 1))
    cnt = (bounds[1:] - bounds[:-1]).reshape(NCORES, NW, 2)
    tiles_needed = -(-cnt // 128)                      # ceil
    TA = tiles_needed[:, :, 0].max(axis=0)
    TB = tiles_needed[:, :, 1].max(axis=0)
    for w in range(NW):
        if TA[w] + TB[w] == 0:
            TA[w] = 1
    TA_tot, TB_tot = int(TA.sum()), int(TB.sum())
    T_tot = TA_tot + TB_tot

    cores = []
    for c in range(NCORES):
        idxA = np.zeros(TA_tot * 128, np.int64)
        idxB = np.zeros(TB_tot * 128, np.int64)
        dstv = np.zeros(T_tot * 128, np.float32)
        dgv = np.zeros(T_tot * 128, np.float32)
        pa = pb = pt = 0
        for w in range(NW):
            for h, (idxarr, tcount, p0) in enumerate(
                ((idxA, TA[w], pa), (idxB, TB[w], pb))
            ):
                g = (c * NW + w) * 2 + h
                s, e = bounds[g], bounds[g + 1]
                n = e - s
                assert n <= tcount * 128
                idxarr[p0 : p0 + n] = idx_s[s:e]
                dstv[pt : pt + n] = dwin_s[s:e]
                dgv[pt : pt + n] = dgi_s[s:e]
                pt += tcount * 128
                if h == 0:
                    pa += tcount * 128
                else:
                    pb += tcount * 128
        cores.append(
            dict(
                gidxA=_wrap_idx(idxA),
                gidxB=_wrap_idx(idxB),
                dstv=_tile_cols(dstv, T_tot),
                dgv=_tile_cols(dgv, T_tot),
            )
        )
    return dict(TA=[int(x) for x in TA], TB=[int(x) for x in TB],
                TA_tot=TA_tot, TB_tot=TB_tot, T_tot=T_tot, cores=cores)


def build_program(TA, TB, TA_tot, TB_tot, T_tot):
    nc = bacc.Bacc("TRN2", target_bir_lowering=False, debug=False,
                   num_devices=NCORES)
    RG = [list(range(NCORES))]

    # ---- DRAM I/O ----
    d_xt = nc.dram_tensor("xt", [2 * 128, PN], F32, kind="ExternalInput")
    d_gidxA = nc.dram_tensor("gidxA", [128, TA_tot * 8], I16, kind="ExternalInput")
    d_gidxB = nc.dram_tensor("gidxB", [128, TB_tot * 8], I16, kind="ExternalInput")
    d_dstv = nc.dram_tensor("dstv", [128, T_tot], F32, kind="ExternalInput")
    d_dgv = nc.dram_tensor("dgv", [128, T_tot], F32, kind="ExternalInput")
    d_iota = nc.dram_tensor("iota", [128, 128], BF16, kind="ExternalInput")
    d_ident = nc.dram_tensor("ident", [128, 128], F32, kind="ExternalInput")
    d_bias = nc.dram_tensor("bias", [128, NBCOL], F32, kind="ExternalInput")
    d_wl = nc.dram_tensor("wl", [L * 256, 256], BF16, kind="ExternalInput")
    d_wr = nc.dram_tensor("wr", [L * 256, 256], F32, kind="ExternalInput")
    d_sk = nc.dram_tensor("sk", [L * 256, 256], F32, kind="ExternalInput")
    d_inw = nc.dram_tensor("inw", [256, 256], F32, kind="ExternalInput")
    d_w1 = nc.dram_tensor("w1", [256, 512], F32, kind="ExternalInput")
    d_w2 = nc.dram_tensor("w2", [512, 256], F32, kind="ExternalInput")
    d_ow = nc.dram_tensor("ow", [256, DOUT], F32, kind="ExternalInput")
    d_perm = nc.dram_tensor("perm", [DOUT, DOUT], F32, kind="ExternalInput")
    d_out = nc.dram_tensor("out", [DOUT, PN], F32, kind="ExternalOutput")

    re_tp = lambda ap: ap.rearrange("(t p) o -> p t o", p=128)

    with tile.TileContext(nc) as tc:
        with (
            tc.tile_pool(name="big", bufs=1) as big,
            tc.tile_pool(name="wts", bufs=1) as wts,
            tc.tile_pool(name="gth", bufs=3) as gth,
            tc.tile_pool(name="sel", bufs=4) as selp,
            tc.tile_pool(name="scr", bufs=3) as scr,
            tc.tile_pool(name="sml", bufs=1) as sml,
            tc.tile_pool(name="psA", bufs=2, space="PSUM") as psA,
            tc.tile_pool(name="psB", bufs=2, space="PSUM") as psB,
            tc.tile_pool(name="psT", bufs=2, space="PSUM") as psT,
            tc.tile_pool(name="dram", bufs=1, space="DRAM") as dram,
        ):
            # persistent SBUF
            X = big.tile([128, 2, PN], F32, name="X")          # h (feature-major)
            Y = big.tile([128, 2, PN], F32, name="Y")          # z / scratch
            dstv = big.tile([128, T_tot], F32, name="dstv")
            dgv = big.tile([128, T_tot], F32, name="dgv")
            iota = big.tile([128, 128], BF16, name="iota")
            ident = big.tile([128, 128], F32, name="ident")
            biases = big.tile([128, NBCOL], F32, name="biases")
            zsum = big.tile([128, 2, NCH], F32, name="zsum")
            zsq = big.tile([128, 2, NCH], F32, name="zsq")
            arpack = big.tile([128, 4], F32, name="arpack")
            arsb = big.tile([128, 4], F32, name="arsb")
            musb = big.tile([128, 2], F32, name="musb")
            varsb = big.tile([128, 2], F32, name="varsb")
            scsb = big.tile([128, 2], F32, name="scsb")
            shsb = big.tile([128, 2], F32, name="shsb")
            tmp2 = big.tile([128, 2], F32, name="tmp2")

            # DRAM internals
            hfull = dram.tile([NPAD, 256], BF16, name="hfull", addr_space="Shared")
            ag_in = dram.tile([PN, 256], BF16, name="ag_in")
            ar_in = dram.tile([128, 4], F32, name="ar_in")
            ar_out = dram.tile([128, 4], F32, name="ar_out")

            gixA = big.tile([128, TA_tot * 8], I16, name="gixA")
            gixB = big.tile([128, TB_tot * 8], I16, name="gixB")
            nc.sync.dma_start(gixA[:], d_gidxA.ap())
            nc.sync.dma_start(gixB[:], d_gidxB.ap())
            nc.sync.dma_start(dstv[:], d_dstv.ap())
            nc.sync.dma_start(dgv[:], d_dgv.ap())
            nc.sync.dma_start(iota[:], d_iota.ap())
            nc.sync.dma_start(ident[:], d_ident.ap())
            nc.sync.dma_start(biases[:], d_bias.ap())

            selstat = None
            if KNOSEL:
                selstat = big.tile([128, 128], BF16, name="selstat")
                nc.vector.tensor_copy(selstat[:], iota[:])

            def bias_ap(nm, lay, pt, npart=128):
                col = BIAS_COLS[(nm, lay if nm in ("bl", "bng", "bnb", "skb") else 0)]
                return biases[0:npart, col + pt : col + pt + 1]

            def tail_transpose_ag(lay):
                """X (feature-major fp32) -> stag bf16 row-major -> ag_in -> AG."""
                stag = big.tile([128, NW, 256], BF16, name=f"stag{lay}",
                                tag="aggbuf")
                for nt in range(NW):
                    for fh in range(2):
                        pst = psT.tile([128, 128], F32, name=f"pst{lay}_{nt}_{fh}",
                                       tag="pst")
                        nc.tensor.transpose(
                            pst[:], X[:, fh, nt * 128 : (nt + 1) * 128], ident[:]
                        )
                        nc.vector.tensor_copy(
                            stag[:, nt, fh * 128 : (fh + 1) * 128], pst[:]
                        )
                nc.sync.dma_start(
                    ag_in.rearrange("(w p) f -> p w f", p=128), stag[:]
                )
                if not KNOAG:
                    nc.gpsimd.collective_compute(
                        "AllGather", mybir.AluOpType.bypass, replica_groups=RG,
                        ins=[ag_in.opt()], outs=[hfull.opt()],
                    )

            # ---------------- phase 0: input projection ----------------
            inw = wts.tile([128, 2, 256], F32, name="inw", tag="wA")
            nc.sync.dma_start(inw[:], re_tp(d_inw.ap()))
            nc.sync.dma_start(Y[:, :, :], d_xt.ap().rearrange("(t p) n -> p t n", p=128))
            for pt in range(2):
                for ci, (s, w) in enumerate(CHUNKS):
                    ps = psB.tile([128, CHUNK], F32, name=f"ps0_{pt}_{ci}", tag="psz")
                    for fi in range(2):
                        nc.tensor.matmul(
                            ps[:, :w], inw[:, fi, pt * 128 : (pt + 1) * 128],
                            Y[:, fi, s : s + w], start=(fi == 0), stop=(fi == 1),
                        )
                    nc.scalar.activation(
                        X[:, pt, s : s + w], ps[:, :w],
                        mybir.ActivationFunctionType.Relu,
                        bias=bias_ap("inb", 0, pt),
                    )
            tail_transpose_ag(-1)

            # ---------------- conv layers ----------------
            for lay in range(L if STAGE >= 1 else 0):
                aggT = big.tile([128, 2, PN], BF16, name=f"aggT{lay}", tag="aggbuf")
                wl = wts.tile([128, 2, 256], BF16, name=f"wl{lay}", tag="wC")
                wr = wts.tile([128, 2, 256], F32, name=f"wr{lay}", tag="wA")
                sk = wts.tile([128, 2, 256], F32, name=f"sk{lay}", tag="wB")
                nc.sync.dma_start(wl[:], re_tp(d_wl.ap()[lay * 256 : (lay + 1) * 256, :]))
                nc.sync.dma_start(wr[:], re_tp(d_wr.ap()[lay * 256 : (lay + 1) * 256, :]))
                nc.sync.dma_start(sk[:], re_tp(d_sk.ap()[lay * 256 : (lay + 1) * 256, :]))

                # --- aggregation ---
                gbufs = {}   # (half, call) -> tile

                def ensure_call(hf, t0, lay=lay):
                    callno = t0 // KBUF
                    if (hf, callno) in gbufs:
                        return gbufs[(hf, callno)]
                    tot = TA_tot if hf == 0 else TB_tot
                    kk = min(KBUF, tot - callno * KBUF)
                    gix = gixA if hf == 0 else gixB
                    gb = gth.tile([128, kk, 256], BF16, name=f"gb{lay}_{hf}_{callno}",
                                  tag=f"gb{hf}")
                    src_ap = hfull[0:HALF, :] if hf == 0 else hfull[HALF:NPAD, :]
                    if not KNOGATHER:
                        nc.gpsimd.dma_gather(gb[:], src_ap,
                                             gix[:, callno * KBUF * 8 : (callno * KBUF + kk) * 8],
                                             kk * 128, kk * 128, 256,
                                             single_packet=False)
                    gbufs[(hf, callno)] = gb
                    return gb

                a = b = t = 0
                for w in range(NW):
                    ntile_w = TA[w] + TB[w]
                    pw = [
                        psA.tile([128, 128], F32, name=f"pw{lay}_{w}_{fh}", tag=f"pw{fh}")
                        for fh in range(2)
                    ]
                    tl = 0
                    for hf, cnt in ((0, TA[w]), (1, TB[w])):
                        for _ in range(cnt):
                            gb = ensure_call(hf, a if hf == 0 else b)
                            r = (a if hf == 0 else b) % KBUF
                            if KNOSEL:
                                sel = selstat
                            else:
                                sel = selp.tile([128, 128], BF16, name=f"sel{lay}_{t}",
                                                tag="sel")
                                nc.vector.tensor_scalar(
                                    sel[:], iota[:], dstv[:, t : t + 1], dgv[:, t : t + 1],
                                    AluOpType.is_equal, AluOpType.mult,
                                )
                            for fh in range(2):
                                nc.tensor.matmul(
                                    pw[fh][:], gb[:, r, fh * 128 : (fh + 1) * 128],
                                    sel[:], start=(tl == 0), stop=(tl == ntile_w - 1),
                                )
                            if hf == 0:
                                a += 1
                            else:
                                b += 1
                            t += 1
                            tl += 1
                    for fh in range(2):
                        nc.vector.tensor_copy(
                            aggT[:, fh, w * 128 : (w + 1) * 128], pw[fh][:]
                        )

                # --- dense z = wl@aggT + wr@hT + bl ; stats ---
                for pt in range(2 if STAGE >= 2 else 0):
                    for ci, (s, w) in enumerate(CHUNKS):
                        ps = psB.tile([128, CHUNK], F32, name=f"psz{lay}_{pt}_{ci}",
                                      tag="psz")
                        for fi in range(2):
                            nc.tensor.matmul(
                                ps[:, :w], wl[:, fi, pt * 128 : (pt + 1) * 128],
                                aggT[:, fi, s : s + w], start=(fi == 0), stop=False,
                            )
                        for fi in range(2):
                            nc.tensor.matmul(
                                ps[:, :w], wr[:, fi, pt * 128 : (pt + 1) * 128],
                                X[:, fi, s : s + w], start=False, stop=(fi == 1),
                            )
                        vw = min(w, max(0, PPC - s))  # valid (non-pad) columns
                        nc.scalar.activation(
                            Y[:, pt, s : s + w], ps[:, :w],
                            mybir.ActivationFunctionType.Identity,
                            bias=bias_ap("bl", lay, pt),
                            accum_out=None,
                        )
                        sq = scr.tile([128, CHUNK], F32, name=f"sq{lay}_{pt}_{ci}",
                                      tag="sq", bufs=6)
                        if vw > 0:
                            nc.scalar.activation(
                                sq[:, :vw], Y[:, pt, s : s + vw],
                                mybir.ActivationFunctionType.Identity,
                                accum_out=zsum[:, pt, ci : ci + 1],
                            )
                            nc.scalar.activation(
                                sq[:, :vw], Y[:, pt, s : s + vw],
                                mybir.ActivationFunctionType.Square,
                                accum_out=zsq[:, pt, ci : ci + 1],
                            )
                        else:
                            nc.vector.memset(zsum[:, pt, ci : ci + 1], 0.0)
                            nc.vector.memset(zsq[:, pt, ci : ci + 1], 0.0)

                for pt in range(2 if STAGE >= 2 else 0):
                    nc.vector.reduce_sum(arpack[:, pt : pt + 1], zsum[:, pt, :],
                                         axis=mybir.AxisListType.X)
                    nc.vector.reduce_sum(arpack[:, 2 + pt : 3 + pt], zsq[:, pt, :],
                                         axis=mybir.AxisListType.X)
                if STAGE >= 2:
                    nc.sync.dma_start(ar_in[:], arpack[:])
                    nc.gpsimd.collective_compute(
                        "AllReduce", mybir.AluOpType.add, replica_groups=RG,
                        ins=[ar_in.opt()], outs=[ar_out.opt()],
                    )
                    nc.sync.dma_start(arsb[:], ar_out[:])

                    # BN scale/shift
                    nc.vector.tensor_scalar_mul(musb[:], arsb[:, 0:2], INV_N)
                    nc.vector.tensor_scalar_mul(varsb[:], arsb[:, 2:4], INV_N)
                    nc.vector.tensor_tensor(tmp2[:], musb[:], musb[:], AluOpType.mult)
                    nc.vector.tensor_tensor(varsb[:], varsb[:], tmp2[:], AluOpType.subtract)
                    nc.vector.tensor_scalar_add(varsb[:], varsb[:], EPS)
                    nc.scalar.sqrt(varsb[:], varsb[:])
                    nc.vector.reciprocal(varsb[:], varsb[:])
                    nc.vector.tensor_tensor(
                        scsb[:], biases[:, BIAS_COLS[("bng", lay)] : BIAS_COLS[("bng", lay)] + 2],
                        varsb[:], AluOpType.mult,
                    )
                    nc.vector.tensor_tensor(tmp2[:], musb[:], scsb[:], AluOpType.mult)
                    nc.vector.tensor_tensor(
                        shsb[:], biases[:, BIAS_COLS[("bnb", lay)] : BIAS_COLS[("bnb", lay)] + 2],
                        tmp2[:], AluOpType.subtract,
                    )

                # skip + normalize + residual add
                for ci, (s, w) in enumerate(CHUNKS if STAGE >= 2 else []):
                    pss = []
                    for pt in range(2):
                        psk = psB.tile([128, CHUNK], F32, name=f"psk{lay}_{pt}_{ci}",
                                       tag="psz")
                        for fi in range(2):
                            nc.tensor.matmul(
                                psk[:, :w], sk[:, fi, pt * 128 : (pt + 1) * 128],
                                X[:, fi, s : s + w], start=(fi == 0), stop=(fi == 1),
                            )
                        pss.append(psk)
                    for pt in range(2):
                        nc.scalar.activation(
                            Y[:, pt, s : s + w], Y[:, pt, s : s + w],
                            mybir.ActivationFunctionType.Relu,
                            bias=shsb[:, pt : pt + 1], scale=scsb[:, pt : pt + 1],
                        )
                        nc.vector.scalar_tensor_tensor(
                            X[:, pt, s : s + w], Y[:, pt, s : s + w],
                            bias_ap("skb", lay, pt), pss[pt][:, :w],
                            AluOpType.add, AluOpType.add,
                        )

                if lay < L - 1 and STAGE >= 2:
                    tail_transpose_ag(lay)

            # ---------------- MLP head + log_softmax ----------------
            w1 = wts.tile([128, 2, 512], F32, name="w1", tag="wA")
            w2 = wts.tile([128, 4, 256], F32, name="w2", tag="wB")
            ow = wts.tile([128, 2, DOUT], F32, name="ow", tag="wC")
            nc.sync.dma_start(w1[:], re_tp(d_w1.ap()))
            nc.sync.dma_start(w2[:], re_tp(d_w2.ap()))
            nc.sync.dma_start(ow[:], re_tp(d_ow.ap()))
            perm = big.tile([DOUT, DOUT], F32, name="perm")
            nc.sync.dma_start(perm[:], d_perm.ap())

            for ci, (s, w) in enumerate(CHUNKS if STAGE >= 3 else []):
                m1 = gth.tile([128, 4, 512], F32, name=f"m1_{ci}", tag="gb0")
                for q in range(4):
                    ps1 = psB.tile([128, CHUNK], F32, name=f"ps1_{ci}_{q}", tag="psz")
                    for fi in range(2):
                        nc.tensor.matmul(
                            ps1[:, :w], w1[:, fi, q * 128 : (q + 1) * 128],
                            X[:, fi, s : s + w], start=(fi == 0), stop=(fi == 1),
                        )
                    nc.scalar.activation(
                        m1[:, q, :w], ps1[:, :w],
                        mybir.ActivationFunctionType.Relu, bias=bias_ap("b1", 0, q),
                    )
                m2 = gth.tile([128, 2, 512], F32, name=f"m2_{ci}", tag="gb1")
                for pt in range(2):
                    ps2 = psB.tile([128, CHUNK], F32, name=f"ps2_{ci}_{pt}", tag="psz")
                    for q in range(4):
                        nc.tensor.matmul(
                            ps2[:, :w], w2[:, q, pt * 128 : (pt + 1) * 128],
                            m1[:, q, :w], start=(q == 0), stop=(q == 3),
                        )
                    nc.scalar.activation(
                        m2[:, pt, :w], ps2[:, :w],
                        mybir.ActivationFunctionType.Identity, bias=bias_ap("b2", 0, pt),
                    )
                if KMLP == "m2":
                    nc.sync.dma_start(d_out.ap()[:, s : s + w], m2[0:DOUT, 0, :w])
                    continue
                psl = psB.tile([DOUT, CHUNK], F32, name=f"psl_{ci}", tag="psz")
                for fi in range(2):
                    nc.tensor.matmul(
                        psl[:, :w], ow[:, fi, :], m2[:, fi, :w],
                        start=(fi == 0), stop=(fi == 1),
                    )
                lg = scr.tile([DOUT, CHUNK], F32, name=f"lg_{ci}", tag="sq", bufs=6)
                nc.scalar.activation(
                    lg[:, :w], psl[:, :w],
                    mybir.ActivationFunctionType.Identity,
                    bias=bias_ap("outb", 0, 0, npart=DOUT),
                )
                if KMLP == "logits":
                    nc.sync.dma_start(d_out.ap()[:, s : s + w], lg[:, :w])
                    continue
                psw = psB.tile([DOUT, CHUNK], F32, name=f"psw_{ci}", tag="psz")
                nc.tensor.matmul(psw[:, :w], perm[:], lg[:, :w], start=True, stop=True)
                if KMLP == "perm":
                    lsw0 = scr.tile([DOUT, CHUNK], F32, name=f"lsw0_{ci}", tag="sq", bufs=6)
                    nc.vector.tensor_copy(lsw0[:, :w], psw[:, :w])
                    nc.sync.dma_start(d_out.ap()[:, s : s + w], lsw0[:, :w])
                    continue
                lsw = scr.tile([DOUT, CHUNK], F32, name=f"lsw_{ci}", tag="sq", bufs=6)
                nc.vector.tensor_copy(lsw[:, :w], psw[:, :w])
                mx = scr.tile([DOUT, CHUNK], F32, name=f"mx_{ci}", tag="sq", bufs=6)
                nc.vector.tensor_tensor(mx[:, :w], lg[:, :w], lsw[:, :w], AluOpType.max)
                nc.vector.tensor_tensor(lg[:, :w], lg[:, :w], mx[:, :w], AluOpType.subtract)
                nc.vector.tensor_tensor(lsw[:, :w], lsw[:, :w], mx[:, :w], AluOpType.subtract)
                if KMLP == "mx":
                    nc.sync.dma_start(d_out.ap()[:, s : s + w], lsw[:, :w])
                    continue
                ex = scr.tile([DOUT, CHUNK], F32, name=f"ex_{ci}", tag="sq", bufs=6)
                nc.scalar.activation(ex[:, :w], lg[:, :w],
                                     mybir.ActivationFunctionType.Exp)
                nc.scalar.activation(lsw[:, :w], lsw[:, :w],
                                     mybir.ActivationFunctionType.Exp)
                if KMLP == "exp":
                    nc.sync.dma_start(d_out.ap()[:, s : s + w], ex[:, :w])
                    continue
                nc.vector.tensor_tensor(ex[:, :w], ex[:, :w], lsw[:, :w], AluOpType.add)
                ln_ = scr.tile([DOUT, CHUNK], F32, name=f"ln_{ci}", tag="sq", bufs=6)
                nc.scalar.activation(ln_[:, :w], ex[:, :w],
                                     mybir.ActivationFunctionType.Ln)
                ot_ = scr.tile([DOUT, CHUNK], F32, name=f"ot_{ci}", tag="sq", bufs=6)
                nc.vector.tensor_tensor(ot_[:, :w], lg[:, :w], ln_[:, :w], AluOpType.subtract)
                nc.sync.dma_start(d_out.ap()[:, s : s + w], ot_[:, :w])

            if STAGE < 3:
                nc.sync.dma_start(d_out.ap(), X[0:DOUT, 0, :])

    nc.compile()
    return nc


_CACHE = {}


def kernel(**inputs):
    inputs = {k: np.asarray(v) for k, v in inputs.items()}
    edge_index = inputs["edge_index"]
    key = hash(edge_index.tobytes())
    if key not in _CACHE:
        plan = plan_edges(edge_index)
        nc = build_program(plan["TA"], plan["TB"], plan["TA_tot"],
                           plan["TB_tot"], plan["T_tot"])
        _CACHE.clear()
        _CACHE[key] = (plan, nc)
    plan, nc = _CACHE[key]

    x = inputs["x"].astype(np.float32)
    # shared (replicated) tensors
    bias = np.zeros((128, NBCOL), np.float32)
    for lay in range(L):
        bias[:, BIAS_COLS[("bl", lay)] : BIAS_COLS[("bl", lay)] + 2] = _pack_vec(inputs["conv_bl"][lay])
        bias[:, BIAS_COLS[("bng", lay)] : BIAS_COLS[("bng", lay)] + 2] = _pack_vec(inputs["bn_g"][lay])
        bias[:, BIAS_COLS[("bnb", lay)] : BIAS_COLS[("bnb", lay)] + 2] = _pack_vec(inputs["bn_b"][lay])
        bias[:, BIAS_COLS[("skb", lay)] : BIAS_COLS[("skb", lay)] + 2] = _pack_vec(inputs["skip_b"][lay])
    bias[:, BIAS_COLS[("inb", 0)] : BIAS_COLS[("inb", 0)] + 2] = _pack_vec(inputs["in_b"])
    b1c = BIAS_COLS[("b1", 0)]
    bias[:, b1c : b1c + 4] = np.asarray(inputs["mlp_b1"], np.float32).reshape(4, 128).T
    bias[:, BIAS_COLS[("b2", 0)] : BIAS_COLS[("b2", 0)] + 2] = _pack_vec(inputs["mlp_b2"])
    bias[0:DOUT, BIAS_COLS[("outb", 0)]] = np.asarray(inputs["out_b"], np.float32)

    shared = dict(
        iota=np.tile(np.arange(128, dtype=np.float32), (128, 1)).astype(ml_dtypes.bfloat16),
        ident=np.eye(128, dtype=np.float32),
        bias=bias,
        wl=np.concatenate([np.ascontiguousarray(inputs["conv_wl"][i].T) for i in range(L)],
                          axis=0).astype(ml_dtypes.bfloat16),
        wr=np.concatenate([np.ascontiguousarray(inputs["conv_wr"][i].T) for i in range(L)],
                          axis=0).astype(np.float32),
        sk=np.concatenate([np.ascontiguousarray(inputs["skip_w"][i].T) for i in range(L)],
                          axis=0).astype(np.float32),
        inw=np.ascontiguousarray(inputs["in_w"].T.astype(np.float32)),
        w1=np.ascontiguousarray(inputs["mlp_w1"].T.astype(np.float32)),
        w2=np.ascontiguousarray(inputs["mlp_w2"].T.astype(np.float32)),
        ow=np.ascontiguousarray(inputs["out_w"].T.astype(np.float32)),
        perm=np.array([[0.0, 1.0], [1.0, 0.0]], np.float32),
    )

    in_maps = []
    for c in range(NCORES):
        xt = np.zeros((256, PN), np.float32)
        xt[:, :PPC] = x[c * PPC : (c + 1) * PPC].T
        m = dict(shared)
        m["xt"] = xt
        m.update(plan["cores"][c])
        in_maps.append(m)

    res = bass_utils.run_bass_kernel_spmd(nc, in_maps, core_ids=list(range(NCORES)))
    out = np.empty((N, DOUT), np.float32)
    for c in range(NCORES):
        out[c * PPC : (c + 1) * PPC] = res.results[c]["out"][:, :PPC].T
    return out



# revision 10
# speedup vs baseline: 10.3055x; 9.2812x over previous
"""Trainium2 Bass kernel for nn_EnhancedSAGEModel (GNN message passing).

Strategy: node-partition across 8 cores (dst-sharding). Each core owns 6250
nodes (padded to 6272 = 49 windows of 128). Per layer:
  - mean-aggregation via dma_gather of h rows (bf16) + one-hot selector
    matmuls on the tensor engine (selector built on-chip by DVE is_equal,
    scaled by 1/deg), accumulating in PSUM (fp32)
  - dense SAGE update + BatchNorm (global stats via AllReduce) + ReLU +
    residual skip, computed feature-major ([256, nodes]) in fp32
  - h_new transposed back to row-major (PE transpose) and AllGathered so
    every core holds the full bf16 h table for the next layer's gather.
Final MLP + log_softmax fused per 512-node chunk.
"""

import os
import sys

sys.path.insert(0, "/opt/trn_rl_repo")

STAGE = int(os.environ.get("KSTAGE", "99"))
KMLP = os.environ.get("KMLP", "full")
NLAYERS = int(os.environ.get("KLAYERS", str(4)))
KNOAG = os.environ.get("KNOAG", "") == "1"
KNOGATHER = os.environ.get("KNOGATHER", "") == "1"
KNOSEL = os.environ.get("KNOSEL", "") == "1"

import numpy as np
import ml_dtypes

import concourse.bass as bass
import concourse.bacc as bacc
import concourse.mybir as mybir
import concourse.tile as tile
from concourse import bass_utils
from concourse.alu_op_type import AluOpType

F32 = mybir.dt.float32
BF16 = mybir.dt.bfloat16
I16 = mybir.dt.int16

N, E, DIN, H, L, DOUT = 50000, 800000, 256, 256, 4, 2
EPS = 1e-5
NCORES = 8
PPC = N // NCORES            # 6250 real nodes per core
NW = 49                      # windows per core
PN = NW * 128                # 6272 padded nodes per core
PAD = PN - PPC               # 22
NPAD = NCORES * PN           # 50176 padded global rows
HALF = NPAD // 2             # 25088
KBUF = 11                    # gather tiles per dma_gather call
CHUNK = 512
CHUNKS = [(i * CHUNK, CHUNK) for i in range(PN // CHUNK)] + [(PN - PN % CHUNK, PN % CHUNK)]
CHUNKS = [(s, w) for (s, w) in CHUNKS if w > 0]
NCH = len(CHUNKS)
INV_N = 1.0 / N

# bias column layout in the packed [128, NBCOL] bias tensor
def _bias_cols():
    cols = {}
    c = 0
    for lay in range(L):
        for nm in ("bl", "bng", "bnb", "skb"):
            cols[(nm, lay)] = c
            c += 2
    cols[("inb", 0)] = c; c += 2
    cols[("b1", 0)] = c; c += 4
    cols[("b2", 0)] = c; c += 2
    cols[("outb", 0)] = c; c += 1
    return cols, c

BIAS_COLS, NBCOL = _bias_cols()


def _wrap_idx(arr):
    """int array -> [128, len/16] int16 wrapped layout, replicated x8."""
    n = len(arr)
    assert n % 16 == 0
    w = arr.reshape(n // 16, 16).T.astype(np.int16)  # [16, n/16]
    return np.tile(w, (8, 1))


def _tile_cols(arr, ntiles):
    """[ntiles*128] -> [128, ntiles], column t = tile t values."""
    return np.ascontiguousarray(arr.reshape(ntiles, 128).T)


def _pack_vec(v):
    """[256] -> [128, 2] (col pt = v[pt*128:(pt+1)*128])."""
    return np.ascontiguousarray(np.asarray(v, np.float32).reshape(2, 128).T)


def plan_edges(edge_index):
    """Build the static per-core aggregation plan from the edge list."""
    src = edge_index[0].astype(np.int64)
    dst = edge_index[1].astype(np.int64)
    deg = np.bincount(dst, minlength=N).astype(np.float64)
    deginv_n = (1.0 / np.clip(deg, 1.0, None)).astype(np.float32)

    core = dst // PPC
    dloc = dst - core * PPC
    win = dloc // 128
    dwin = dloc % 128
    gsrc = src + PAD * (src // PPC)       # position in padded global table
    half = (gsrc >= HALF).astype(np.int64)
    idxval = gsrc - half * HALF
    dgi = deginv_n[dst]

    order = np.lexsort((src, half, win, core))
    core_s, win_s, half_s = core[order], win[order], half[order]
    idx_s, dwin_s, dgi_s = idxval[order], dwin[order], dgi[order]

    # group boundaries for (core, win, half)
    key = (core_s * NW + win_s) * 2 + half_s
    bounds = np.searchsorted(key, np.arange(NCORES * NW * 2 + 1))
    cnt = (bounds[1:] - bounds[:-1]).reshape(NCORES, NW, 2)
    tiles_needed = -(-cnt // 128)                      # ceil
    TA = tiles_needed[:, :, 0].max(axis=0)
    TB = tiles_needed[:, :, 1].max(axis=0)
    for w in range(NW):
        if TA[w] + TB[w] == 0:
            TA[w] = 1
    TA_tot, TB_tot = int(TA.sum()), int(TB.sum())
    T_tot = TA_tot + TB_tot

    cores = []
    for c in range(NCORES):
        idxA = np.zeros(TA_tot * 128, np.int64)
        idxB = np.zeros(TB_tot * 128, np.int64)
        dstv = np.zeros(T_tot * 128, np.float32)
        dgv = np.zeros(T_tot * 128, np.float32)
        pa = pb = pt = 0
        for w in range(NW):
            for h, (idxarr, tcount, p0) in enumerate(
                ((idxA, TA[w], pa), (idxB, TB[w], pb))
            ):
                g = (c * NW + w) * 2 + h
                s, e = bounds[g], bounds[g + 1]
                n = e - s
                assert n <= tcount * 128
                idxarr[p0 : p0 + n] = idx_s[s:e]
                dstv[pt : pt + n] = dwin_s[s:e]
                dgv[pt : pt + n] = dgi_s[s:e]
                pt += tcount * 128
                if h == 0:
                    pa += tcount * 128
                else:
                    pb += tcount * 128
        cores.append(
            dict(
                gidxA=_wrap_idx(idxA),
                gidxB=_wrap_idx(idxB),
                dstv=_tile_cols(dstv, T_tot),
                dgv=_tile_cols(dgv, T_tot),
            )
        )
    return dict(TA=[int(x) for x in TA], TB=[int(x) for x in TB],
                TA_tot=TA_tot, TB_tot=TB_tot, T_tot=T_tot, cores=cores)


def build_program(TA, TB, TA_tot, TB_tot, T_tot):
    nc = bacc.Bacc("TRN2", target_bir_lowering=False, debug=False,
                   num_devices=NCORES)
    RG = [list(range(NCORES))]

    # ---- DRAM I/O ----
    d_xt = nc.dram_tensor("xt", [2 * 128, PN], F32, kind="ExternalInput")
    d_gidxA = nc.dram_tensor("gidxA", [128, TA_tot * 8], I16, kind="ExternalInput")
    d_gidxB = nc.dram_tensor("gidxB", [128, TB_tot * 8], I16, kind="ExternalInput")
    d_dstv = nc.dram_tensor("dstv", [128, T_tot], F32, kind="ExternalInput")
    d_dgv = nc.dram_tensor("dgv", [128, T_tot], F32, kind="ExternalInput")
    d_iota = nc.dram_tensor("iota", [128, 128], BF16, kind="ExternalInput")
    d_ident = nc.dram_tensor("ident", [128, 128], F32, kind="ExternalInput")
    d_bias = nc.dram_tensor("bias", [128, NBCOL], F32, kind="ExternalInput")
    d_wl = nc.dram_tensor("wl", [L * 256, 256], BF16, kind="ExternalInput")
    d_wr = nc.dram_tensor("wr", [L * 256, 256], F32, kind="ExternalInput")
    d_sk = nc.dram_tensor("sk", [L * 256, 256], F32, kind="ExternalInput")
    d_inw = nc.dram_tensor("inw", [256, 256], F32, kind="ExternalInput")
    d_w1 = nc.dram_tensor("w1", [256, 512], F32, kind="ExternalInput")
    d_w2 = nc.dram_tensor("w2", [512, 256], F32, kind="ExternalInput")
    d_ow = nc.dram_tensor("ow", [256, DOUT], F32, kind="ExternalInput")
    d_perm = nc.dram_tensor("perm", [DOUT, DOUT], F32, kind="ExternalInput")
    d_out = nc.dram_tensor("out", [DOUT, PN], F32, kind="ExternalOutput")

    re_tp = lambda ap: ap.rearrange("(t p) o -> p t o", p=128)

    with tile.TileContext(nc) as tc:
        with (
            tc.tile_pool(name="big", bufs=1) as big,
            tc.tile_pool(name="wts", bufs=1) as wts,
            tc.tile_pool(name="gth", bufs=3) as gth,
            tc.tile_pool(name="sel", bufs=4) as selp,
            tc.tile_pool(name="scr", bufs=3) as scr,
            tc.tile_pool(name="sml", bufs=1) as sml,
            tc.tile_pool(name="psA", bufs=2, space="PSUM") as psA,
            tc.tile_pool(name="psB", bufs=2, space="PSUM") as psB,
            tc.tile_pool(name="psT", bufs=2, space="PSUM") as psT,
            tc.tile_pool(name="dram", bufs=1, space="DRAM") as dram,
        ):
            # persistent SBUF
            X = big.tile([128, 2, PN], F32, name="X")          # h (feature-major)
            Y = big.tile([128, 2, PN], F32, name="Y")          # z / scratch
            dstv = big.tile([128, T_tot], F32, name="dstv")
            dgv = big.tile([128, T_tot], F32, name="dgv")
            iota = big.tile([128, 128], BF16, name="iota")
            ident = big.tile([128, 128], F32, name="ident")
            biases = big.tile([128, NBCOL], F32, name="biases")
            zsum = big.tile([128, 2, NCH], F32, name="zsum")
            zsq = big.tile([128, 2, NCH], F32, name="zsq")
            arpack = big.tile([128, 4], F32, name="arpack")
            arsb = big.tile([128, 4], F32, name="arsb")
            musb = big.tile([128, 2], F32, name="musb")
            varsb = big.tile([128, 2], F32, name="varsb")
            scsb = big.tile([128, 2], F32, name="scsb")
            shsb = big.tile([128, 2], F32, name="shsb")
            tmp2 = big.tile([128, 2], F32, name="tmp2")

            # DRAM internals — one hfull per AllGather instance (Shared
            # scratchpads are single-writer)
            hsp = "Local" if KNOAG else "Shared"
            hfulls = [
                dram.tile([NPAD, 256], BF16, name=f"hfull{i}", addr_space=hsp)
                for i in range(L)
            ]
            ag_in = dram.tile([PN, 256], BF16, name="ag_in")
            ar_in = dram.tile([128, 4], F32, name="ar_in")
            ar_out = dram.tile([128, 4], F32, name="ar_out")

            gixA = big.tile([128, TA_tot * 8], I16, name="gixA")
            gixB = big.tile([128, TB_tot * 8], I16, name="gixB")
            nc.sync.dma_start(gixA[:], d_gidxA.ap())
            nc.sync.dma_start(gixB[:], d_gidxB.ap())
            nc.sync.dma_start(dstv[:], d_dstv.ap())
            nc.sync.dma_start(dgv[:], d_dgv.ap())
            nc.sync.dma_start(iota[:], d_iota.ap())
            nc.sync.dma_start(ident[:], d_ident.ap())
            nc.sync.dma_start(biases[:], d_bias.ap())

            selstat = None
            if KNOSEL:
                selstat = big.tile([128, 128], BF16, name="selstat")
                nc.vector.tensor_copy(selstat[:], iota[:])

            def bias_ap(nm, lay, pt, npart=128):
                col = BIAS_COLS[(nm, lay if nm in ("bl", "bng", "bnb", "skb") else 0)]
                return biases[0:npart, col + pt : col + pt + 1]

            def tail_transpose_ag(lay):
                """X (feature-major fp32) -> stag bf16 row-major -> ag_in -> AG."""
                stag = big.tile([128, NW, 256], BF16, name=f"stag{lay}",
                                tag="aggbuf")
                for nt in range(NW):
                    for fh in range(2):
                        pst = psT.tile([128, 128], F32, name=f"pst{lay}_{nt}_{fh}",
                                       tag="pst")
                        nc.tensor.transpose(
                            pst[:], X[:, fh, nt * 128 : (nt + 1) * 128], ident[:]
                        )
                        nc.vector.tensor_copy(
                            stag[:, nt, fh * 128 : (fh + 1) * 128], pst[:]
                        )
                nc.sync.dma_start(
                    ag_in.rearrange("(w p) f -> p w f", p=128), stag[:]
                )
                if not KNOAG:
                    nc.gpsimd.collective_compute(
                        "AllGather", mybir.AluOpType.bypass, replica_groups=RG,
                        ins=[ag_in.opt()], outs=[hfulls[lay + 1].opt()],
                    )

            # ---------------- phase 0: input projection ----------------
            inw = wts.tile([128, 2, 256], F32, name="inw", tag="wA")
            nc.sync.dma_start(inw[:], re_tp(d_inw.ap()))
            nc.sync.dma_start(Y[:, :, :], d_xt.ap().rearrange("(t p) n -> p t n", p=128))
            for pt in range(2):
                for ci, (s, w) in enumerate(CHUNKS):
                    ps = psB.tile([128, CHUNK], F32, name=f"ps0_{pt}_{ci}", tag="psz")
                    for fi in range(2):
                        nc.tensor.matmul(
                            ps[:, :w], inw[:, fi, pt * 128 : (pt + 1) * 128],
                            Y[:, fi, s : s + w], start=(fi == 0), stop=(fi == 1),
                        )
                    nc.scalar.activation(
                        X[:, pt, s : s + w], ps[:, :w],
                        mybir.ActivationFunctionType.Relu,
                        bias=bias_ap("inb", 0, pt),
                    )
            tail_transpose_ag(-1)

            # ---------------- conv layers ----------------
            for lay in range(L if STAGE >= 1 else 0):
                aggT = big.tile([128, 2, PN], BF16, name=f"aggT{lay}", tag="aggbuf")
                wl = wts.tile([128, 2, 256], BF16, name=f"wl{lay}", tag="wC")
                wr = wts.tile([128, 2, 256], F32, name=f"wr{lay}", tag="wA")
                sk = wts.tile([128, 2, 256], F32, name=f"sk{lay}", tag="wB")
                nc.sync.dma_start(wl[:], re_tp(d_wl.ap()[lay * 256 : (lay + 1) * 256, :]))
                nc.sync.dma_start(wr[:], re_tp(d_wr.ap()[lay * 256 : (lay + 1) * 256, :]))
                nc.sync.dma_start(sk[:], re_tp(d_sk.ap()[lay * 256 : (lay + 1) * 256, :]))

                # --- aggregation ---
                gbufs = {}   # (half, call) -> tile

                def ensure_call(hf, t0, lay=lay):
                    callno = t0 // KBUF
                    if (hf, callno) in gbufs:
                        return gbufs[(hf, callno)]
                    tot = TA_tot if hf == 0 else TB_tot
                    kk = min(KBUF, tot - callno * KBUF)
                    gix = gixA if hf == 0 else gixB
                    gb = gth.tile([128, kk, 256], BF16, name=f"gb{lay}_{hf}_{callno}",
                                  tag=f"gb{hf}")
                    hfull = hfulls[lay]
                    src_ap = hfull[0:HALF, :] if hf == 0 else hfull[HALF:NPAD, :]
                    if KNOGATHER:
                        # linear DMA of the same volume (isolates the
                        # scatter/SWDGE cost vs plain streaming)
                        nc.sync.dma_start(
                            gb[:],
                            hfull[callno * KBUF * 128 : (callno * KBUF + kk) * 128, :]
                            .rearrange("(k p) f -> p k f", p=128),
                        )
                    else:
                        nc.gpsimd.dma_gather(gb[:], src_ap,
                                             gix[:, callno * KBUF * 8 : (callno * KBUF + kk) * 8],
                                             kk * 128, kk * 128, 256,
                                             single_packet=False)
                    gbufs[(hf, callno)] = gb
                    return gb

                a = b = t = 0
                for w in range(NW):
                    ntile_w = TA[w] + TB[w]
                    pw = [
                        psA.tile([128, 128], F32, name=f"pw{lay}_{w}_{fh}", tag=f"pw{fh}")
                        for fh in range(2)
                    ]
                    tl = 0
                    for hf, cnt in ((0, TA[w]), (1, TB[w])):
                        for _ in range(cnt):
                            gb = ensure_call(hf, a if hf == 0 else b)
                            r = (a if hf == 0 else b) % KBUF
                            if KNOSEL:
                                sel = selstat
                            else:
                                sel = selp.tile([128, 128], BF16, name=f"sel{lay}_{t}",
                                                tag="sel")
                                nc.vector.tensor_scalar(
                                    sel[:], iota[:], dstv[:, t : t + 1], dgv[:, t : t + 1],
                                    AluOpType.is_equal, AluOpType.mult,
                                )
                            for fh in range(2):
                                nc.tensor.matmul(
                                    pw[fh][:], gb[:, r, fh * 128 : (fh + 1) * 128],
                                    sel[:], start=(tl == 0), stop=(tl == ntile_w - 1),
                                )
                            if hf == 0:
                                a += 1
                            else:
                                b += 1
                            t += 1
                            tl += 1
                    for fh in range(2):
                        nc.vector.tensor_copy(
                            aggT[:, fh, w * 128 : (w + 1) * 128], pw[fh][:]
                        )

                # --- dense z = wl@aggT + wr@hT + bl ; stats ---
                for pt in range(2 if STAGE >= 2 else 0):
                    for ci, (s, w) in enumerate(CHUNKS):
                        ps = psB.tile([128, CHUNK], F32, name=f"psz{lay}_{pt}_{ci}",
                                      tag="psz")
                        for fi in range(2):
                            nc.tensor.matmul(
                                ps[:, :w], wl[:, fi, pt * 128 : (pt + 1) * 128],
                                aggT[:, fi, s : s + w], start=(fi == 0), stop=False,
                            )
                        for fi in range(2):
                            nc.tensor.matmul(
                                ps[:, :w], wr[:, fi, pt * 128 : (pt + 1) * 128],
                                X[:, fi, s : s + w], start=False, stop=(fi == 1),
                            )
                        vw = min(w, max(0, PPC - s))  # valid (non-pad) columns
                        nc.scalar.activation(
                            Y[:, pt, s : s + w], ps[:, :w],
                            mybir.ActivationFunctionType.Identity,
                            bias=bias_ap("bl", lay, pt),
                            accum_out=None,
                        )
                        sq = scr.tile([128, CHUNK], F32, name=f"sq{lay}_{pt}_{ci}",
                                      tag="sq", bufs=6)
                        if vw > 0:
                            nc.scalar.activation(
                                sq[:, :vw], Y[:, pt, s : s + vw],
                                mybir.ActivationFunctionType.Identity,
                                accum_out=zsum[:, pt, ci : ci + 1],
                            )
                            nc.scalar.activation(
                                sq[:, :vw], Y[:, pt, s : s + vw],
                                mybir.ActivationFunctionType.Square,
                                accum_out=zsq[:, pt, ci : ci + 1],
                            )
                        else:
                            nc.vector.memset(zsum[:, pt, ci : ci + 1], 0.0)
                            nc.vector.memset(zsq[:, pt, ci : ci + 1], 0.0)

                for pt in range(2 if STAGE >= 2 else 0):
                    nc.vector.reduce_sum(arpack[:, pt : pt + 1], zsum[:, pt, :],
                                         axis=mybir.AxisListType.X)
                    nc.vector.reduce_sum(arpack[:, 2 + pt : 3 + pt], zsq[:, pt, :],
                                         axis=mybir.AxisListType.X)
                if STAGE >= 2:
                    nc.sync.dma_start(ar_in[:], arpack[:])
                    nc.gpsimd.collective_compute(
                        "AllReduce", mybir.AluOpType.add, replica_groups=RG,
                        ins=[ar_in.opt()], outs=[ar_out.opt()],
                    )
                    nc.sync.dma_start(arsb[:], ar_out[:])

                    # BN scale/shift
                    nc.vector.tensor_scalar_mul(musb[:], arsb[:, 0:2], INV_N)
                    nc.vector.tensor_scalar_mul(varsb[:], arsb[:, 2:4], INV_N)
                    nc.vector.tensor_tensor(tmp2[:], musb[:], musb[:], AluOpType.mult)
                    nc.vector.tensor_tensor(varsb[:], varsb[:], tmp2[:], AluOpType.subtract)
                    nc.vector.tensor_scalar_add(varsb[:], varsb[:], EPS)
                    nc.scalar.sqrt(varsb[:], varsb[:])
                    nc.vector.reciprocal(varsb[:], varsb[:])
                    nc.vector.tensor_tensor(
                        scsb[:], biases[:, BIAS_COLS[("bng", lay)] : BIAS_COLS[("bng", lay)] + 2],
                        varsb[:], AluOpType.mult,
                    )
                    nc.vector.tensor_tensor(tmp2[:], musb[:], scsb[:], AluOpType.mult)
                    nc.vector.tensor_tensor(
                        shsb[:], biases[:, BIAS_COLS[("bnb", lay)] : BIAS_COLS[("bnb", lay)] + 2],
                        tmp2[:], AluOpType.subtract,
                    )

                # skip + normalize + residual add
                for ci, (s, w) in enumerate(CHUNKS if STAGE >= 2 else []):
                    pss = []
                    for pt in range(2):
                        psk = psB.tile([128, CHUNK], F32, name=f"psk{lay}_{pt}_{ci}",
                                       tag="psz")
                        for fi in range(2):
                            nc.tensor.matmul(
                                psk[:, :w], sk[:, fi, pt * 128 : (pt + 1) * 128],
                                X[:, fi, s : s + w], start=(fi == 0), stop=(fi == 1),
                            )
                        pss.append(psk)
                    for pt in range(2):
                        nc.scalar.activation(
                            Y[:, pt, s : s + w], Y[:, pt, s : s + w],
                            mybir.ActivationFunctionType.Relu,
                            bias=shsb[:, pt : pt + 1], scale=scsb[:, pt : pt + 1],
                        )
                        nc.vector.scalar_tensor_tensor(
                            X[:, pt, s : s + w], Y[:, pt, s : s + w],
                            bias_ap("skb", lay, pt), pss[pt][:, :w],
                            AluOpType.add, AluOpType.add,
                        )

                if lay < L - 1 and STAGE >= 2:
                    tail_transpose_ag(lay)

            # ---------------- MLP head + log_softmax ----------------
            w1 = wts.tile([128, 2, 512], F32, name="w1", tag="wA")
            w2 = wts.tile([128, 4, 256], F32, name="w2", tag="wB")
            ow = wts.tile([128, 2, DOUT], F32, name="ow", tag="wC")
            nc.sync.dma_start(w1[:], re_tp(d_w1.ap()))
            nc.sync.dma_start(w2[:], re_tp(d_w2.ap()))
            nc.sync.dma_start(ow[:], re_tp(d_ow.ap()))
            perm = big.tile([DOUT, DOUT], F32, name="perm")
            nc.sync.dma_start(perm[:], d_perm.ap())

            for ci, (s, w) in enumerate(CHUNKS if STAGE >= 3 else []):
                m1 = gth.tile([128, 4, 512], F32, name=f"m1_{ci}", tag="gb0")
                for q in range(4):
                    ps1 = psB.tile([128, CHUNK], F32, name=f"ps1_{ci}_{q}", tag="psz")
                    for fi in range(2):
                        nc.tensor.matmul(
                            ps1[:, :w], w1[:, fi, q * 128 : (q + 1) * 128],
                            X[:, fi, s : s + w], start=(fi == 0), stop=(fi == 1),
                        )
                    nc.scalar.activation(
                        m1[:, q, :w], ps1[:, :w],
                        mybir.ActivationFunctionType.Relu, bias=bias_ap("b1", 0, q),
                    )
                m2 = gth.tile([128, 2, 512], F32, name=f"m2_{ci}", tag="gb1")
                for pt in range(2):
                    ps2 = psB.tile([128, CHUNK], F32, name=f"ps2_{ci}_{pt}", tag="psz")
                    for q in range(4):
                        nc.tensor.matmul(
                            ps2[:, :w], w2[:, q, pt * 128 : (pt + 1) * 128],
                            m1[:, q, :w], start=(q == 0), stop=(q == 3),
                        )
                    nc.scalar.activation(
                        m2[:, pt, :w], ps2[:, :w],
                        mybir.ActivationFunctionType.Identity, bias=bias_ap("b2", 0, pt),
                    )
                if KMLP == "m2":
                    nc.sync.dma_start(d_out.ap()[:, s : s + w], m2[0:DOUT, 0, :w])
                    continue
                psl = psB.tile([DOUT, CHUNK], F32, name=f"psl_{ci}", tag="psz")
                for fi in range(2):
                    nc.tensor.matmul(
                        psl[:, :w], ow[:, fi, :], m2[:, fi, :w],
                        start=(fi == 0), stop=(fi == 1),
                    )
                lg = scr.tile([DOUT, CHUNK], F32, name=f"lg_{ci}", tag="sq", bufs=6)
                nc.scalar.activation(
                    lg[:, :w], psl[:, :w],
                    mybir.ActivationFunctionType.Identity,
                    bias=bias_ap("outb", 0, 0, npart=DOUT),
                )
                if KMLP == "logits":
                    nc.sync.dma_start(d_out.ap()[:, s : s + w], lg[:, :w])
                    continue
                psw = psB.tile([DOUT, CHUNK], F32, name=f"psw_{ci}", tag="psz")
                nc.tensor.matmul(psw[:, :w], perm[:], lg[:, :w], start=True, stop=True)
                if KMLP == "perm":
                    lsw0 = scr.tile([DOUT, CHUNK], F32, name=f"lsw0_{ci}", tag="sq", bufs=6)
                    nc.vector.tensor_copy(lsw0[:, :w], psw[:, :w])
                    nc.sync.dma_start(d_out.ap()[:, s : s + w], lsw0[:, :w])
                    continue
                lsw = scr.tile([DOUT, CHUNK], F32, name=f"lsw_{ci}", tag="sq", bufs=6)
                nc.vector.tensor_copy(lsw[:, :w], psw[:, :w])
                mx = scr.tile([DOUT, CHUNK], F32, name=f"mx_{ci}", tag="sq", bufs=6)
                nc.vector.tensor_tensor(mx[:, :w], lg[:, :w], lsw[:, :w], AluOpType.max)
                nc.vector.tensor_tensor(lg[:, :w], lg[:, :w], mx[:, :w], AluOpType.subtract)
                nc.vector.tensor_tensor(lsw[:, :w], lsw[:, :w], mx[:, :w], AluOpType.subtract)
                if KMLP == "mx":
                    nc.sync.dma_start(d_out.ap()[:, s : s + w], lsw[:, :w])
                    continue
                ex = scr.tile([DOUT, CHUNK], F32, name=f"ex_{ci}", tag="sq", bufs=6)
                nc.scalar.activation(ex[:, :w], lg[:, :w],
                                     mybir.ActivationFunctionType.Exp)
                nc.scalar.activation(lsw[:, :w], lsw[:, :w],
                                     mybir.ActivationFunctionType.Exp)
                if KMLP == "exp":
                    nc.sync.dma_start(d_out.ap()[:, s : s + w], ex[:, :w])
                    continue
                nc.vector.tensor_tensor(ex[:, :w], ex[:, :w], lsw[:, :w], AluOpType.add)
                ln_ = scr.tile([DOUT, CHUNK], F32, name=f"ln_{ci}", tag="sq", bufs=6)
                nc.scalar.activation(ln_[:, :w], ex[:, :w],
                                     mybir.ActivationFunctionType.Ln)
                ot_ = scr.tile([DOUT, CHUNK], F32, name=f"ot_{ci}", tag="sq", bufs=6)
                nc.vector.tensor_tensor(ot_[:, :w], lg[:, :w], ln_[:, :w], AluOpType.subtract)
                nc.sync.dma_start(d_out.ap()[:, s : s + w], ot_[:, :w])

            if STAGE < 3:
                nc.sync.dma_start(d_out.ap(), X[0:DOUT, 0, :])

    nc.compile()
    return nc


_CACHE = {}


def kernel(**inputs):
    inputs = {k: np.asarray(v) for k, v in inputs.items()}
    edge_index = inputs["edge_index"]
    key = hash(edge_index.tobytes())
    if key not in _CACHE:
        plan = plan_edges(edge_index)
        nc = build_program(plan["TA"], plan["TB"], plan["TA_tot"],
                           plan["TB_tot"], plan["T_tot"])
        _CACHE.clear()
        _CACHE[key] = (plan, nc)
    plan, nc = _CACHE[key]

    x = inputs["x"].astype(np.float32)
    # shared (replicated) tensors
    bias = np.zeros((128, NBCOL), np.float32)
    for lay in range(L):
        bias[:, BIAS_COLS[("bl", lay)] : BIAS_COLS[("bl", lay)] + 2] = _pack_vec(inputs["conv_bl"][lay])
        bias[:, BIAS_COLS[("bng", lay)] : BIAS_COLS[("bng", lay)] + 2] = _pack_vec(inputs["bn_g"][lay])
        bias[:, BIAS_COLS[("bnb", lay)] : BIAS_COLS[("bnb", lay)] + 2] = _pack_vec(inputs["bn_b"][lay])
        bias[:, BIAS_COLS[("skb", lay)] : BIAS_COLS[("skb", lay)] + 2] = _pack_vec(inputs["skip_b"][lay])
    bias[:, BIAS_COLS[("inb", 0)] : BIAS_COLS[("inb", 0)] + 2] = _pack_vec(inputs["in_b"])
    b1c = BIAS_COLS[("b1", 0)]
    bias[:, b1c : b1c + 4] = np.asarray(inputs["mlp_b1"], np.float32).reshape(4, 128).T
    bias[:, BIAS_COLS[("b2", 0)] : BIAS_COLS[("b2", 0)] + 2] = _pack_vec(inputs["mlp_b2"])
    bias[0:DOUT, BIAS_COLS[("outb", 0)]] = np.asarray(inputs["out_b"], np.float32)

    shared = dict(
        iota=np.tile(np.arange(128, dtype=np.float32), (128, 1)).astype(ml_dtypes.bfloat16),
        ident=np.eye(128, dtype=np.float32),
        bias=bias,
        wl=np.concatenate([np.ascontiguousarray(inputs["conv_wl"][i].T) for i in range(L)],
                          axis=0).astype(ml_dtypes.bfloat16),
        wr=np.concatenate([np.ascontiguousarray(inputs["conv_wr"][i].T) for i in range(L)],
                          axis=0).astype(np.float32),
        sk=np.concatenate([np.ascontiguousarray(inputs["skip_w"][i].T) for i in range(L)],
                          axis=0).astype(np.float32),
        inw=np.ascontiguousarray(inputs["in_w"].T.astype(np.float32)),
        w1=np.ascontiguousarray(inputs["mlp_w1"].T.astype(np.float32)),
        w2=np.ascontiguousarray(inputs["mlp_w2"].T.astype(np.float32)),
        ow=np.ascontiguousarray(inputs["out_w"].T.astype(np.float32)),
        perm=np.array([[0.0, 1.0], [1.0, 0.0]], np.float32),
    )

    in_maps = []
    for c in range(NCORES):
        xt = np.zeros((256, PN), np.float32)
        xt[:, :PPC] = x[c * PPC : (c + 1) * PPC].T
        m = dict(shared)
        m["xt"] = xt
        m.update(plan["cores"][c])
        in_maps.append(m)

    res = bass_utils.run_bass_kernel_spmd(nc, in_maps, core_ids=list(range(NCORES)))
    out = np.empty((N, DOUT), np.float32)
    for c in range(NCORES):
        out[c * PPC : (c + 1) * PPC] = res.results[c]["out"][:, :PPC].T
    return out

